# revision 1
# baseline (speedup 1.0000x reference)
"""Trainium2 Bass kernel for batched ADMM-TV (mode='only_tv'), 8 cores.

Math restructuring (transposed layout, columns = samples; es = eta/g,
ts = tau/a, t = lam/g):  with  na_k = -(D x_{k-1} + es_{k-1}),
nb_k = -(x_{k-1} + ts_{k-1}),  cna = clamp(na, +-t),  rb = relu(nb),
CR_k = [cna_k; rb_k]  (128 rows stacked), the whole reference iteration
collapses to one elementwise clamp plus a few small matmuls:

    CR_k      = clip(ANB_k, lo, hi)           (one DVE tensor_scalar)
    x_k       = [M4|H] XT_{k-1} + N CR_k      (PSUM-accumulated matmuls)
    ANB_{k+1} = [W1|W3] XT_{k-1} + W2 CR_k

where XT = [x; tgt] stacked 128 rows and M4, N, H, W1..W3 are
host-precomputed 64/128-wide constant matrices (fp32r on device).

The shipped variant ("raw2") additionally materializes x into SBUF only
on even iterations (depth-2 substitution; see _build_graph_raw2), so the
ACT-copy round trip spans two iterations and the steady state is bound
by the DVE clamp throughput.  All synchronization is hand-placed
semaphores on a raw Bacc graph - the Tile-framework version of the same
dataflow (_build_graph) measures ~3x slower on hardware due to semaphore
handoff overhead.

Data parallel: batch 4096 -> 8 cores x 512 samples -> 2 column-chunks of
256 per core for cross-engine pipelining.  Measured: ~255 us on silicon
(245/256/265 us across three runs of the For_i-amplified paired-delta
bench), rel err 1.23e-3 vs the fp32 reference (fp32r matmul rounding).
"""

import numpy as np

_GAMMA = 2.0
_ALPHA = 5.0
_LAM = 1e-4
_N_ITER = 200
_B = 4096
_K = 64
_NCORES = 8
_BLOC = _B // _NCORES  # 512
_NCHUNK = 2
_F = _BLOC // _NCHUNK  # 256

_cache = {}


def _host_matrices():
    I = np.eye(_K, dtype=np.float64)
    Delta = np.roll(I, 1, axis=1) - I
    return Delta, I


def _build_graph(reps=None):
    """Build the compute graph.  reps=None -> production graph.
    reps=R -> benchmark variant: the whole solve (init + N_ITER
    iterations) is wrapped in a hardware For_i loop executed R times so
    device time can be measured through wall-clock deltas."""
    import concourse.bacc as bacc
    import concourse.tile as tile
    from concourse import mybir

    f32 = mybir.dt.float32
    f32r = mybir.dt.float32r
    Alu = mybir.AluOpType

    nc = bacc.Bacc("TRN2", target_bir_lowering=False, debug=False,
                   num_devices=_NCORES)

    tgt = nc.dram_tensor("tgt", [_K, _BLOC], f32r, kind="ExternalInput").ap()
    x0 = nc.dram_tensor("x0", [_K, _BLOC], f32r, kind="ExternalInput").ap()
    wA = nc.dram_tensor("wA", [128, 128], f32r, kind="ExternalInput").ap()
    wB = nc.dram_tensor("wB", [128, 128], f32r, kind="ExternalInput").ap()
    wC = nc.dram_tensor("wC", [128, 64], f32r, kind="ExternalInput").ap()
    wD = nc.dram_tensor("wD", [128, 64], f32r, kind="ExternalInput").ap()
    wI = nc.dram_tensor("wI", [64, 128], f32r, kind="ExternalInput").ap()
    slo = nc.dram_tensor("slo", [128, 1], f32, kind="ExternalInput").ap()
    shi = nc.dram_tensor("shi", [128, 1], f32, kind="ExternalInput").ap()
    out = nc.dram_tensor("out", [_K, _BLOC], f32r, kind="ExternalOutput").ap()

    with tile.TileContext(nc) as tc:
        with tc.tile_pool(name="consts", bufs=1) as consts, \
             tc.tile_pool(name="xts", bufs=1) as xts, \
             tc.tile_pool(name="crs", bufs=2) as crs, \
             tc.tile_pool(name="panb", bufs=2, space="PSUM") as panb, \
             tc.tile_pool(name="px", bufs=2, space="PSUM") as px:

            def load_const(name, src, shape, dt_=f32r):
                t = consts.tile(shape, dt_, name=name, tag=name)
                nc.sync.dma_start(t[:], src[:])
                return t

            wA_sb = load_const("wA_sb", wA, [128, 128])
            wB_sb = load_const("wB_sb", wB, [128, 128])
            wC_sb = load_const("wC_sb", wC, [128, 64])
            wD_sb = load_const("wD_sb", wD, [128, 64])
            wI_sb = load_const("wI_sb", wI, [64, 128])
            slo_sb = load_const("slo_sb", slo, [128, 1], dt_=f32)
            shi_sb = load_const("shi_sb", shi, [128, 1], dt_=f32)

            xt = []   # xt[c][j]: [128,F], rows 0:64 = x, rows 64:128 = tg
            for c in range(_NCHUNK):
                sl = slice(c * _F, (c + 1) * _F)
                t0 = xts.tile([128, _F], f32r, name=f"xt0_{c}", tag=f"xt0_{c}")
                t1 = xts.tile([128, _F], f32r, name=f"xt1_{c}", tag=f"xt1_{c}")
                nc.sync.dma_start(t0[64:128, :], tgt[:, sl])
                nc.sync.dma_start(t1[64:128, :], tgt[:, sl])
                xt.append((t0, t1))

            import contextlib
            loop_cm = (tc.For_i(0, reps, 1) if reps is not None
                       else contextlib.nullcontext())
            with loop_cm:
                anb = []  # current ANB psum tile per chunk
                for c in range(_NCHUNK):
                    sl = slice(c * _F, (c + 1) * _F)
                    nc.sync.dma_start(xt[c][0][0:64, :], x0[:, sl])
                    a = panb.tile([128, _F], f32, name=f"anb_{c}",
                                  tag=f"anb_{c}")
                    nc.tensor.matmul(a[:], wI_sb[:], xt[c][0][0:64, :],
                                     start=True, stop=True)
                    anb.append(a)
                _loop_body(nc, tc, crs, panb, px, xt, anb,
                           wA_sb, wB_sb, wC_sb, wD_sb, slo_sb, shi_sb,
                           f32, f32r, Alu)

            for c in range(_NCHUNK):
                sl = slice(c * _F, (c + 1) * _F)
                nc.sync.dma_start(out[:, sl], xt[c][_N_ITER % 2][0:64, :])

    nc.compile()
    return nc


def _loop_body(nc, tc, crs, panb, px, xt, anb, wA_sb, wB_sb, wC_sb, wD_sb,
               slo_sb, shi_sb, f32, f32r, Alu):
    for k in range(1, _N_ITER + 1):
                crk = []
                for c in range(_NCHUNK):
                    cr = crs.tile([128, _F], f32r, name=f"cr_{c}", tag=f"cr_{c}")
                    nc.vector.tensor_scalar(cr[:], anb[c][:], slo_sb[:],
                                            shi_sb[:], op0=Alu.max,
                                            op1=Alu.min)
                    crk.append(cr)
                # cr-dependent matmuls lead each PSUM group so the PE can
                # start them the moment cr lands (the prev-x-dependent
                # matmuls wait on the previous iteration's ACT copy).
                xbk = []
                for c in range(_NCHUNK):
                    cr = crk[c]
                    if k < _N_ITER:
                        a2 = panb.tile([128, _F], f32, name=f"anb_{c}",
                                       tag=f"anb_{c}")
                        nc.tensor.matmul(a2[:], wB_sb[:], cr[:],
                                         start=True, stop=False)
                        anb[c] = a2
                    xb = px.tile([64, _F], f32, name=f"xb_{c}", tag=f"xb_{c}")
                    nc.tensor.matmul(xb[:], wD_sb[:], cr[:],
                                     start=True, stop=False)
                    xbk.append(xb)
                for c in range(_NCHUNK):
                    prev = xt[c][(k - 1) % 2]
                    nc.tensor.matmul(xbk[c][:], wC_sb[:], prev[:],
                                     start=False, stop=True)
                    if k < _N_ITER:
                        nc.tensor.matmul(anb[c][:], wA_sb[:], prev[:],
                                         start=False, stop=True)
                for c in range(_NCHUNK):
                    cur = xt[c][k % 2]
                    nc.scalar.copy(cur[0:64, :], xbk[c][:])


def _build_graph_raw(reps=None):
    """Raw Bacc builder with hand-placed semaphores (no Tile framework).

    Engine streams per iteration k (per chunk c in {A,B}):
      DVE : ts_c(k)    = clamp/relu(anb_c[k%2])       -> cr_c      inc sem_cr[c]=k
      PE  : mmB_c(k)   = wB @ cr_c  (start)  -> anb_c[(k+1)%2]
            mmD_c(k)   = wD @ cr_c  (start)  -> xb_c[k%2]
            mmC_c(k)   = wC @ xt_c[(k-1)%2] (stop) -> xb_c[k%2]    inc sem_xb[c]=k
            mmA_c(k)   = wA @ xt_c[(k-1)%2] (stop) -> anb_c[(k+1)%2] inc sem_anb[c]=k+1
      ACT : act_c(k)   = copy xb_c[k%2] -> xt_c[k%2][0:64]         inc sem_x[c]=k

    Waits (everything else is implied by per-engine program order):
      ts_c(k)  waits sem_anb[c] >= k
      mmB_c(k) waits sem_cr[c]  >= k
      mmC_c(k) waits sem_x[c]   >= k-1
      act_c(k) waits sem_xb[c]  >= k
    """
    import contextlib

    import concourse.bacc as bacc
    from concourse import mybir

    f32 = mybir.dt.float32
    f32r = mybir.dt.float32r
    Alu = mybir.AluOpType

    nc = bacc.Bacc("TRN2", target_bir_lowering=False, debug=False,
                   num_devices=_NCORES)

    tgt = nc.dram_tensor("tgt", [_K, _BLOC], f32r, kind="ExternalInput").ap()
    x0 = nc.dram_tensor("x0", [_K, _BLOC], f32r, kind="ExternalInput").ap()
    wA = nc.dram_tensor("wA", [128, 128], f32r, kind="ExternalInput").ap()
    wB = nc.dram_tensor("wB", [128, 128], f32r, kind="ExternalInput").ap()
    wC = nc.dram_tensor("wC", [128, 64], f32r, kind="ExternalInput").ap()
    wD = nc.dram_tensor("wD", [128, 64], f32r, kind="ExternalInput").ap()
    wI = nc.dram_tensor("wI", [64, 128], f32r, kind="ExternalInput").ap()
    slo = nc.dram_tensor("slo", [128, 1], f32, kind="ExternalInput").ap()
    shi = nc.dram_tensor("shi", [128, 1], f32, kind="ExternalInput").ap()
    out = nc.dram_tensor("out", [_K, _BLOC], f32r, kind="ExternalOutput").ap()

    wA_sb = nc.alloc_sbuf_tensor("wA_sb", [128, 128], f32r).ap()
    wB_sb = nc.alloc_sbuf_tensor("wB_sb", [128, 128], f32r).ap()
    wC_sb = nc.alloc_sbuf_tensor("wC_sb", [128, 64], f32r).ap()
    wD_sb = nc.alloc_sbuf_tensor("wD_sb", [128, 64], f32r).ap()
    wI_sb = nc.alloc_sbuf_tensor("wI_sb", [64, 128], f32r).ap()
    slo_sb = nc.alloc_sbuf_tensor("slo_sb", [128, 1], f32).ap()
    shi_sb = nc.alloc_sbuf_tensor("shi_sb", [128, 1], f32).ap()
    xt = [[nc.alloc_sbuf_tensor(f"xt{j}_{c}", [128, _F], f32r).ap()
           for j in range(2)] for c in range(_NCHUNK)]
    cr = [[nc.alloc_sbuf_tensor(f"cr{j}_{c}", [128, _F], f32r).ap()
           for j in range(2)] for c in range(_NCHUNK)]
    anb = [[nc.alloc_psum_tensor(f"anb{j}_{c}", [128, _F], f32).ap()
            for j in range(2)] for c in range(_NCHUNK)]
    xb = [[nc.alloc_psum_tensor(f"xb{j}_{c}", [64, _F], f32).ap()
           for j in range(2)] for c in range(_NCHUNK)]

    sem_dma = nc.alloc_semaphore("sem_dma")
    sem_cr = [nc.alloc_semaphore(f"sem_cr_{c}") for c in range(_NCHUNK)]
    sem_anb = [nc.alloc_semaphore(f"sem_anb_{c}") for c in range(_NCHUNK)]
    sem_xb = [nc.alloc_semaphore(f"sem_xb_{c}") for c in range(_NCHUNK)]
    sem_x = [nc.alloc_semaphore(f"sem_x_{c}") for c in range(_NCHUNK)]

    sem_outer = nc.alloc_semaphore("sem_outer")

    # ---- one-time DMAs (weights, targets, clamp consts)
    n_outer = 0
    for c in range(_NCHUNK):
        sl = slice(c * _F, (c + 1) * _F)
        nc.sync.dma_start(xt[c][0][64:128, :], tgt[:, sl]).then_inc(sem_outer, 16)
        nc.sync.dma_start(xt[c][1][64:128, :], tgt[:, sl]).then_inc(sem_outer, 16)
        n_outer += 2
    for t, src in ((wA_sb, wA), (wB_sb, wB), (wC_sb, wC), (wD_sb, wD),
                   (wI_sb, wI), (slo_sb, slo), (shi_sb, shi)):
        nc.sync.dma_start(t[:], src[:]).then_inc(sem_outer, 16)
        n_outer += 1

    clear_sems = [sem_dma] + sem_cr + sem_anb + sem_xb + sem_x

    import contextlib

    @contextlib.contextmanager
    def _rep_loop():
        if reps is None:
            yield
            return
        with nc.Fori(0, reps):
            nc.all_engine_barrier()
            for s in clear_sems:
                nc.gpsimd.sem_clear(s)
            nc.all_engine_barrier()
            yield

    with _rep_loop():
        _emit_raw_body(nc, x0, xt, cr, anb, xb, wA_sb, wB_sb, wC_sb, wD_sb,
                       wI_sb, slo_sb, shi_sb, sem_dma, sem_outer, n_outer,
                       sem_cr, sem_anb, sem_xb, sem_x, Alu)

    # ---- output
    for c in range(_NCHUNK):
        sl = slice(c * _F, (c + 1) * _F)
        nc.sync.dma_start(out[:, sl], xt[c][_N_ITER % 2][0:64, :],
                          )._wait_ge(sem_x[c], _N_ITER).then_inc(sem_outer, 16)

    nc.compile()
    return nc


def _emit_raw_body(nc, x0, xt, cr, anb, xb, wA_sb, wB_sb, wC_sb, wD_sb,
                   wI_sb, slo_sb, shi_sb, sem_dma, sem_outer, n_outer,
                   sem_cr, sem_anb, sem_xb, sem_x, Alu):
    # per-rep init: x0 load + anb_1 = [-Delta; -I] @ x0
    for c in range(_NCHUNK):
        sl = slice(c * _F, (c + 1) * _F)
        dma = nc.sync.dma_start(xt[c][0][0:64, :], x0[:, sl])
        if c == 0:
            dma._wait_ge(sem_outer, 16 * n_outer)
        dma.then_inc(sem_dma, 16)
    for c in range(_NCHUNK):
        mm = nc.tensor.matmul(anb[c][1][:], wI_sb[:], xt[c][0][0:64, :],
                              start=True, stop=True)
        if c == 0:
            mm._wait_ge(sem_dma, 32)
        mm.then_inc(sem_anb[c])

    # ---- main loop
    # Per chunk c the ops are: ts (DVE), then on PE mmA/mmC (x-dep group
    # starts, gated on the previous ACT copy), mmB/mmD (cr-dep group
    # stops), then the ACT copy.  PE work is chunk-grouped so one chunk's
    # stall never blocks the other's critical matmuls, and the chunk
    # visit order alternates per iteration to balance the two chains.
    def emit_pe(nc, c, k, kb, nb, pb, phase):
        # Cross-chunk interleaved PE order: phase 0 = mmD (cr-dep xb
        # start), 1 = mmA (x-dep anb start), 2 = mmB (cr-dep anb stop,
        # gates next ts), 3 = mmC (x-dep xb stop, gates ACT copy).
        if phase == 0:
            nc.tensor.matmul(xb[c][kb][:], wD_sb[:], cr[c][kb][:],
                             start=True, stop=False, skip_group_check=True,
                             )._wait_ge(sem_cr[c], k)
        elif phase == 1 and k < _N_ITER:
            mm = nc.tensor.matmul(anb[c][nb][:], wA_sb[:], xt[c][pb][:],
                                  start=True, stop=False,
                                  skip_group_check=True)
            if k > 1:
                mm._wait_ge(sem_x[c], k - 1)
            else:
                mm._wait_ge(sem_dma, 32)
        elif phase == 2 and k < _N_ITER:
            nc.tensor.matmul(anb[c][nb][:], wB_sb[:], cr[c][kb][:],
                             start=False, stop=True, skip_group_check=True,
                             ).then_inc(sem_anb[c])
        elif phase == 3:
            mm = nc.tensor.matmul(xb[c][kb][:], wC_sb[:], xt[c][pb][:],
                                  start=False, stop=True,
                                  skip_group_check=True)
            if k == _N_ITER:
                mm._wait_ge(sem_x[c], k - 1)
            mm.then_inc(sem_xb[c])

    for k in range(1, _N_ITER + 1):
        kb = k % 2          # bank holding ANB_k / xb_k / xt_k / cr_k
        nb = (k + 1) % 2    # bank for ANB_{k+1}
        pb = (k - 1) % 2    # bank holding xt_{k-1}
        order = list(range(_NCHUNK))
        for c in order:
            nc.vector.tensor_scalar(
                cr[c][kb][:], anb[c][kb][:], slo_sb[:], shi_sb[:],
                op0=Alu.max, op1=Alu.min,
            )._wait_ge(sem_anb[c], k).then_inc(sem_cr[c])
        for c in order:
            for phase in range(4):
                emit_pe(nc, c, k, kb, nb, pb, phase)
        for c in order:
            nc.scalar.copy(xt[c][kb][0:64, :], xb[c][kb][:],
                           )._wait_ge(sem_xb[c], k).then_inc(sem_x[c])




def _build_graph_raw2(reps=None):
    """Depth-2 x-materialization variant of the raw builder: the SBUF copy
    of x (ACT) happens only on even iterations, so the ACT round-trip
    spans two iterations and the steady state is DVE-bound.

    Odd k :  ANB_{k+1} = [W1|W3] XT_{k-1} + W2 cr_k            (2 mm)
    Even k:  x_k       = [M4^2|(I+M4)H] XT_{k-2} + M4N cr_{k-1} + N cr_k
             ANB_{k+1} = [U1x|U1t] XT_{k-2} + U2 cr_{k-1} + W2 cr_k
                                                               (6 mm + ACT)
    """
    import contextlib

    import concourse.bacc as bacc
    from concourse import mybir

    f32 = mybir.dt.float32
    f32r = mybir.dt.float32r
    Alu = mybir.AluOpType

    nc = bacc.Bacc("TRN2", target_bir_lowering=False, debug=False,
                   num_devices=_NCORES)

    tgt = nc.dram_tensor("tgt", [_K, _BLOC], f32r, kind="ExternalInput").ap()
    x0 = nc.dram_tensor("x0", [_K, _BLOC], f32r, kind="ExternalInput").ap()
    names = [("wA", [128, 128]), ("wB", [128, 128]), ("wD", [128, 64]),
             ("wI", [64, 128]), ("wU1", [128, 128]), ("wU2", [128, 128]),
             ("wP2", [128, 64]), ("wMN", [128, 64])]
    dram_w = {n: nc.dram_tensor(n, s, f32r, kind="ExternalInput").ap()
              for n, s in names}
    slo = nc.dram_tensor("slo", [128, 1], f32, kind="ExternalInput").ap()
    shi = nc.dram_tensor("shi", [128, 1], f32, kind="ExternalInput").ap()
    out = nc.dram_tensor("out", [_K, _BLOC], f32r, kind="ExternalOutput").ap()

    w_sb = {n: nc.alloc_sbuf_tensor(n + "_sb", s, f32r).ap()
            for n, s in names}
    slo_sb = nc.alloc_sbuf_tensor("slo_sb", [128, 1], f32).ap()
    shi_sb = nc.alloc_sbuf_tensor("shi_sb", [128, 1], f32).ap()
    xt = [[nc.alloc_sbuf_tensor(f"xt{j}_{c}", [128, _F], f32r).ap()
           for j in range(2)] for c in range(_NCHUNK)]
    cr = [[nc.alloc_sbuf_tensor(f"cr{j}_{c}", [128, _F], f32r).ap()
           for j in range(3)] for c in range(_NCHUNK)]
    anb = [[nc.alloc_psum_tensor(f"anb{j}_{c}", [128, _F], f32).ap()
            for j in range(2)] for c in range(_NCHUNK)]
    xb = [nc.alloc_psum_tensor(f"xb_{c}", [64, _F], f32).ap()
          for c in range(_NCHUNK)]

    sem_dma = nc.alloc_semaphore("sem_dma")
    sem_outer = nc.alloc_semaphore("sem_outer")
    sem_cr = [nc.alloc_semaphore(f"sem_cr_{c}") for c in range(_NCHUNK)]
    sem_anb = [nc.alloc_semaphore(f"sem_anb_{c}") for c in range(_NCHUNK)]
    sem_xb = [nc.alloc_semaphore(f"sem_xb_{c}") for c in range(_NCHUNK)]
    sem_x = [nc.alloc_semaphore(f"sem_x_{c}") for c in range(_NCHUNK)]

    n_outer = 0
    for c in range(_NCHUNK):
        sl = slice(c * _F, (c + 1) * _F)
        nc.sync.dma_start(xt[c][0][64:128, :], tgt[:, sl]).then_inc(sem_outer, 16)
        nc.sync.dma_start(xt[c][1][64:128, :], tgt[:, sl]).then_inc(sem_outer, 16)
        n_outer += 2
    for n, _s in names:
        nc.sync.dma_start(w_sb[n][:], dram_w[n][:]).then_inc(sem_outer, 16)
        n_outer += 1
    for t, srcap in ((slo_sb, slo), (shi_sb, shi)):
        nc.sync.dma_start(t[:], srcap[:]).then_inc(sem_outer, 16)
        n_outer += 1

    clear_sems = [sem_dma] + sem_cr + sem_anb + sem_xb + sem_x

    @contextlib.contextmanager
    def _rep_loop():
        if reps is None:
            yield
            return
        with nc.Fori(0, reps):
            nc.all_engine_barrier()
            for s in clear_sems:
                nc.gpsimd.sem_clear(s)
            nc.all_engine_barrier()
            yield

    with _rep_loop():
        # per-rep init: x0 -> XT[0];  ANB_1 = [-Delta;-I] @ x0
        for c in range(_NCHUNK):
            sl = slice(c * _F, (c + 1) * _F)
            dma = nc.sync.dma_start(xt[c][0][0:64, :], x0[:, sl])
            if c == 0:
                dma._wait_ge(sem_outer, 16 * n_outer)
            dma.then_inc(sem_dma, 16)
        for c in range(_NCHUNK):
            mm = nc.tensor.matmul(anb[c][1][:], w_sb["wI"][:],
                                  xt[c][0][0:64, :], start=True, stop=True)
            if c == 0:
                mm._wait_ge(sem_dma, 32)
            mm.then_inc(sem_anb[c])

        for k in range(1, _N_ITER + 1):
            kb = k % 2
            nb = (k + 1) % 2
            k3 = k % 3               # cr buffer (triple-buffered so the
            p3 = (k - 1) % 3         # wB hoist below stays WAR-safe)
            j = k // 2               # materialization index at even k
            for c in range(_NCHUNK):
                nc.vector.tensor_scalar(
                    cr[c][k3][:], anb[c][kb][:], slo_sb[:], shi_sb[:],
                    op0=Alu.max, op1=Alu.min,
                )._wait_ge(sem_anb[c], k).then_inc(sem_cr[c])
            for c in range(_NCHUNK):
                if k % 2 == 1:
                    # odd: anb group only, reads XT_{k-1} = xt[(j) % 2]
                    xsrc = xt[c][j % 2]
                    mm = nc.tensor.matmul(anb[c][nb][:], w_sb["wA"][:],
                                          xsrc[:], start=True, stop=False,
                                          skip_group_check=True)
                    if k > 1:
                        mm._wait_ge(sem_x[c], j)
                    else:
                        mm._wait_ge(sem_dma, 32)
                    nc.tensor.matmul(anb[c][nb][:], w_sb["wB"][:],
                                     cr[c][k3][:], start=False, stop=True,
                                     skip_group_check=True,
                                     )._wait_ge(sem_cr[c], k,
                                                ).then_inc(sem_anb[c])
                else:
                    # even: anb (depth-2) with wB hoisted ahead of the
                    # act-path (xb) matmuls: the next ts unblocks after
                    # three matmuls instead of five; the xb group has two
                    # iterations of slack.
                    xsrc = xt[c][(j - 1) % 2]
                    crp = cr[c][p3]      # cr_{k-1}
                    crk = cr[c][k3]
                    if k < _N_ITER:
                        mm = nc.tensor.matmul(anb[c][nb][:], w_sb["wU1"][:],
                                              xsrc[:], start=True, stop=False,
                                              skip_group_check=True)
                        mm._wait_ge(sem_x[c], j - 1)
                        nc.tensor.matmul(anb[c][nb][:], w_sb["wU2"][:],
                                         crp[:], start=False, stop=False,
                                         skip_group_check=True)
                        nc.tensor.matmul(anb[c][nb][:], w_sb["wB"][:],
                                         crk[:], start=False, stop=True,
                                         skip_group_check=True,
                                         )._wait_ge(sem_cr[c], k,
                                                    ).then_inc(sem_anb[c])
                        nc.tensor.matmul(xb[c][:], w_sb["wP2"][:], xsrc[:],
                                         start=True, stop=False,
                                         skip_group_check=True)
                        nc.tensor.matmul(xb[c][:], w_sb["wMN"][:], crp[:],
                                         start=False, stop=False,
                                         skip_group_check=True)
                        nc.tensor.matmul(xb[c][:], w_sb["wD"][:], crk[:],
                                         start=False, stop=True,
                                         skip_group_check=True,
                                         ).then_inc(sem_xb[c])
                    else:
                        mm = nc.tensor.matmul(xb[c][:], w_sb["wP2"][:],
                                              xsrc[:], start=True, stop=False,
                                              skip_group_check=True)
                        mm._wait_ge(sem_x[c], j - 1)
                        nc.tensor.matmul(xb[c][:], w_sb["wMN"][:], crp[:],
                                         start=False, stop=False,
                                         skip_group_check=True)
                        nc.tensor.matmul(xb[c][:], w_sb["wD"][:], crk[:],
                                         start=False, stop=True,
                                         skip_group_check=True,
                                         )._wait_ge(sem_cr[c], k,
                                                    ).then_inc(sem_xb[c])
            if k % 2 == 0:
                for c in range(_NCHUNK):
                    nc.scalar.copy(xt[c][j % 2][0:64, :], xb[c][:],
                                   )._wait_ge(sem_xb[c], j).then_inc(sem_x[c])

    for c in range(_NCHUNK):
        sl = slice(c * _F, (c + 1) * _F)
        nc.sync.dma_start(out[:, sl], xt[c][(_N_ITER // 2) % 2][0:64, :],
                          )._wait_ge(sem_x[c], _N_ITER // 2,
                                     ).then_inc(sem_outer, 16)

    nc.compile()
    return nc


_VARIANT = "raw2"


def _get_graph():
    if "nc" not in _cache:
        builders = {"tile": _build_graph, "raw": _build_graph_raw,
                    "raw2": _build_graph_raw2}
        _cache["nc"] = builders[_VARIANT]()
    return _cache["nc"]


def kernel(target, A, x0):
    target = np.ascontiguousarray(target, dtype=np.float32)
    A = np.ascontiguousarray(A, dtype=np.float32)
    x0 = np.ascontiguousarray(x0, dtype=np.float32)

    # ---- host-side precompute of the tiny constant matrices (f64 -> f32)
    Delta, I = _host_matrices()
    Af = A.astype(np.float64)
    G = Af.T @ Af
    inv_item = np.linalg.inv(G + _GAMMA * (Delta.T @ Delta) + _ALPHA * I)
    M2 = _GAMMA * inv_item @ Delta
    M3 = _ALPHA * inv_item
    M4 = M2 @ Delta + M3
    H = inv_item @ G
    N = np.hstack([M2, M3])           # 64 x 128
    S1 = np.vstack([Delta, I])        # 128 x 64
    W1 = S1 - 2.0 * S1 @ M4           # 128 x 64
    W2 = np.eye(128) - 2.0 * S1 @ N   # 128 x 128
    W3 = -2.0 * S1 @ H                # 128 x 64

    f = np.float32
    MA = np.hstack([W1, W3])          # 128 x 128, acts on [x; tg]
    lhs_wA = np.ascontiguousarray(MA.T, dtype=f)
    lhs_wB = np.ascontiguousarray(W2.T, dtype=f)
    MC = np.hstack([M4, H])           # 64 x 128
    lhs_wC = np.ascontiguousarray(MC.T, dtype=f)
    lhs_wD = np.ascontiguousarray(N.T, dtype=f)
    lhs_wI = np.ascontiguousarray((-S1).T, dtype=f)
    # depth-2 matrices
    P2x = M4 @ M4
    P2t = (I + M4) @ H
    MN = M4 @ N
    lhs_wP2 = np.ascontiguousarray(np.hstack([P2x, P2t]).T, dtype=f)
    lhs_wMN = np.ascontiguousarray(MN.T, dtype=f)
    lhs_wU1 = np.ascontiguousarray(
        np.hstack([S1 @ (M4 - 2.0 * P2x), S1 @ (H - 2.0 * P2t)]).T, dtype=f)
    lhs_wU2 = np.ascontiguousarray((S1 @ N - 2.0 * S1 @ MN).T, dtype=f)

    t = _LAM / _GAMMA
    slo = np.concatenate([np.full(64, -t), np.zeros(64)]).astype(f)[:, None]
    shi = np.concatenate([np.full(64, t), np.full(64, 3.0e38)]).astype(f)[:, None]
    slo = np.ascontiguousarray(slo)
    shi = np.ascontiguousarray(shi)

    nc = _get_graph()

    in_maps = []
    for i in range(_NCORES):
        rows = slice(i * _BLOC, (i + 1) * _BLOC)
        in_maps.append({
            "tgt": np.ascontiguousarray(target[rows].T),
            "x0": np.ascontiguousarray(x0[rows].T),
            "wA": lhs_wA,
            "wB": lhs_wB,
            "wC": lhs_wC,
            "wD": lhs_wD,
            "wI": lhs_wI,
            "wP2": lhs_wP2,
            "wMN": lhs_wMN,
            "wU1": lhs_wU1,
            "wU2": lhs_wU2,
            "slo": slo,
            "shi": shi,
        })

    from concourse.bass_utils import run_bass_kernel_spmd

    res = run_bass_kernel_spmd(nc, in_maps, core_ids=list(range(_NCORES)))
    outs = [r["out"] for r in res.results]
    full = np.concatenate([o.T for o in outs], axis=0)
    return np.ascontiguousarray(full, dtype=np.float32)

